# revision 35
# baseline (speedup 1.0000x reference)
"""Two-block single-head transformer (B=4, S=4096, E=256) on 8 TRN2 NeuronCores.

Sharding: core c -> batch b=c//2, query-half h=c%2 (2048 query rows each).
Each core receives its batch's x ROLLED so that its own query rows are always
rows [0:2048] -- this keeps the on-device program identical across cores
(pure SPMD, no partition-id branching).  Attention is permutation-invariant
over keys, so layer-1 may use the rolled key order.  Layer-2 keys come from a
pairwise AllGather of the LN1 outputs in canonical order.

Math per layer (matches torch reference):
  q/k/v = x @ W.T + b ; att = softmax((q k^T)/sqrt(S)) ; o = att v
  layernorm over E with gamma/beta.

On-device layout: scores are computed transposed (keys on partitions,
queries on free dim) so the exp'd scores tile feeds straight into the
att @ V matmul as the stationary operand.  A ones-column appended to V makes
the softmax denominator fall out of the same accumulation.

The layer boundary is pipelined: the AllGather is split into one chunk per
512-query block, and the layer-2 transposes/projections are chunked behind
those, so they overlap the remaining layer-1 attention instead of
serializing after it.
"""

import sys

sys.path.insert(0, "/opt/trn_rl_repo")

import numpy as np
import ml_dtypes

import concourse.bass as bass
import concourse.tile as tile
from concourse import bacc, mybir
from concourse import bass_utils

f32 = mybir.dt.float32
bf16 = mybir.dt.bfloat16

B, S, E = 4, 4096, 256
P = 128
SQ = S // 2          # query rows per core
DC = E // P          # 2 chunks of the head dim
KC = S // P          # 32 key chunks
QB = 512             # query block (matmul moving dim)
NQB = SQ // QB       # 4 query blocks per core
QS = QB // P         # 4 query sub-blocks per block
N_CORES = 8
EPS = 1e-5
SCALE = 1.0 / np.sqrt(np.float32(S))

_COMPILED = None


def _broadcast_ap(vec_ap, parts, n):
    """[n] DRAM vector -> [parts, n] partition-broadcast access pattern."""
    return bass.AP(
        tensor=vec_ap.tensor,
        offset=vec_ap.offset,
        ap=[[0, parts], [1, n]],
    )


def _build():
    nc = bacc.Bacc(
        "TRN2", target_bir_lowering=False, debug=False, num_devices=N_CORES
    )

    # --- kernel I/O (per core) ---
    xT = nc.dram_tensor("xT", [P, DC * S], bf16, kind="ExternalInput").ap()
    xq = nc.dram_tensor("xq", [SQ, E], f32, kind="ExternalInput").ap()
    wts = {
        n: nc.dram_tensor(n, [P, DC * E], bf16, kind="ExternalInput").ap()
        for n in ["wqt1", "wkt1", "wvt1", "wqt2", "wkt2", "wvt2"]
    }
    vecs = {
        n: nc.dram_tensor(n, [E], f32, kind="ExternalInput").ap()
        for n in ["bq1", "bk1", "bv1", "bq2", "bk2", "bv2",
                  "g1", "be1", "g2", "be2"]
    }
    y = nc.dram_tensor("y", [SQ, E], f32, kind="ExternalOutput").ap()

    with tile.TileContext(nc) as tc:
        _emit(nc, tc, xT, xq, wts, vecs, y)

    nc.compile()
    return nc


def _emit(nc, tc, xT, xq, wts, vecs, y):
    from contextlib import ExitStack

    ctx = ExitStack()
    with ctx:
        const = ctx.enter_context(tc.tile_pool(name="const", bufs=1))
        srcT_pool = ctx.enter_context(tc.tile_pool(name="srcT", bufs=1))
        kt_pool = ctx.enter_context(tc.tile_pool(name="kt", bufs=2))
        v_pool = ctx.enter_context(tc.tile_pool(name="v", bufs=2))
        qt_pool = ctx.enter_context(tc.tile_pool(name="qt", bufs=2))
        o_pool = ctx.enter_context(tc.tile_pool(name="okeep", bufs=16))
        work = ctx.enter_context(tc.tile_pool(name="work", bufs=4))
        expp = ctx.enter_context(tc.tile_pool(name="expp", bufs=4))
        stats = ctx.enter_context(tc.tile_pool(name="stats", bufs=8))
        dram = ctx.enter_context(tc.tile_pool(name="dram", bufs=1, space="DRAM"))
        # scores and projection matmuls share the same 4 [128,512] PSUM banks
        mm_ps = ctx.enter_context(tc.tile_pool(name="mm_ps", bufs=3, space="PSUM"))
        p2_ps = ctx.enter_context(tc.tile_pool(name="p2_ps", bufs=1, space="PSUM"))
        o_ps = ctx.enter_context(tc.tile_pool(name="o_ps", bufs=4, space="PSUM"))

        # --- constants (sync HWDGE queue, in order of first use) ---
        w_sb, bias_sb, bcast_sb = {}, {}, {}

        def _load_w(n):
            t = const.tile([P, DC, E], bf16, tag=f"w_{n}", name=f"w_{n}")
            nc.sync.dma_start(
                out=t[:], in_=wts[n].rearrange("p (dc o) -> p dc o", dc=DC)
            )
            w_sb[n] = t

        def _load_b(n):
            t = const.tile([P, DC], f32, tag=f"b_{n}", name=f"b_{n}")
            nc.sync.dma_start(
                out=t[:], in_=vecs[n].rearrange("(dc p) -> p dc", p=P)
            )
            bias_sb[n] = t

        def _load_bc(n):
            t = const.tile([P, E], f32, tag=f"bc_{n}", name=f"bc_{n}")
            nc.sync.dma_start(out=t[:], in_=_broadcast_ap(vecs[n], P, E))
            bcast_sb[n] = t

        # need-ordered on the sync queue: layer-1 K/V path first, then the
        # rest behind the first xT chunks (emitted in the layer-1 section).
        for n in ["wkt1", "wvt1", "wqt1"]:
            _load_w(n)
        _load_b("bk1")
        _load_b("bq1")
        _load_bc("bv1")

        def _load_late_consts():
            for n in ["g1", "be1"]:
                _load_bc(n)
            for n in ["wqt2", "wkt2", "wvt2"]:
                _load_w(n)
            for n in ["bq2", "bk2"]:
                _load_b(n)
            for n in ["bv2", "g2", "be2"]:
                _load_bc(n)

        o_bounce = dram.tile([SQ, E], bf16)
        # one contiguous AllGather output per query block:
        # rows [0:QB] = own-half canonical rows qb*QB..,
        # rows [QB:2QB] = other-half canonical rows SQ+qb*QB..
        o_chunks = [
            dram.tile([2 * QB, E], bf16, name=f"agchunk{i}") for i in range(NQB)
        ]

        def proj_k_chunk(kT_sb, srcT_sb, wk, bk, dst_c, src_c, pool=None):
            """KT[:, :, dst_c*QB:+QB] from srcT columns [src_c*QB:+QB]."""
            pool = pool or mm_ps
            for oc in range(DC):
                ps = pool.tile([P, QB], f32, tag="mm", name="pk")
                for dc in range(DC):
                    nc.tensor.matmul(
                        ps[:],
                        lhsT=wk[:, dc, oc * P:(oc + 1) * P],
                        rhs=srcT_sb[:, dc, src_c * QB:(src_c + 1) * QB],
                        start=(dc == 0),
                        stop=(dc == DC - 1),
                    )
                nc.vector.tensor_scalar_add(
                    out=kT_sb[:, oc, dst_c * QB:(dst_c + 1) * QB],
                    in0=ps[:],
                    scalar1=bk[:, oc:oc + 1],
                )

        def proj_v_chunk(v_sb, srcT_sb, wv, bv_bc, dst_c, src_c, pool=None):
            """V rows [dst_c*QB : +QB] (4 sub-chunks of 128) from srcT."""
            pool = pool or mm_ps
            for i in range(QB // P):
                sc_dst = dst_c * (QB // P) + i
                sc_src = src_c * (QB // P) + i
                ps = pool.tile([P, QB], f32, tag="mm", name="pv")
                for dc in range(DC):
                    nc.tensor.matmul(
                        ps[:, :E],
                        lhsT=srcT_sb[:, dc, sc_src * P:(sc_src + 1) * P],
                        rhs=wv[:, dc, :],
                        start=(dc == 0),
                        stop=(dc == DC - 1),
                    )
                nc.vector.tensor_add(
                    out=v_sb[:, sc_dst, :E], in0=ps[:, :E], in1=bv_bc[:]
                )

        def proj_q_chunk(qT_out, srcT_sb, wq, bq, qc, pool=None):
            pool = pool or mm_ps
            for oc in range(DC):
                ps = pool.tile([P, QB], f32, tag="mm", name="pq")
                for dc in range(DC):
                    nc.tensor.matmul(
                        ps[:],
                        lhsT=wq[:, dc, oc * P:(oc + 1) * P],
                        rhs=srcT_sb[:, dc, qc * QB:(qc + 1) * QB],
                        start=(dc == 0),
                        stop=(dc == DC - 1),
                    )
                nc.vector.tensor_scalar_add(
                    out=qT_out[:, oc, qc * QB:(qc + 1) * QB],
                    in0=ps[:],
                    scalar1=bq[:, oc:oc + 1],
                )

        def attention_block(kT_sb, v_sb, qT_sb, qb, resid_tiles, g_bc, be_bc,
                            out_cb, kc_order=None, act_rsqrt=False,
                            mid_emit=None, out_dst=None):
            """One 512-query attention block + residual + layernorm.

            The layernorm rstd is computed with a DVE-only Newton rsqrt so the
            Scalar engine never leaves the Exp activation table.
            """
            if kc_order is None:
                kc_order = list(range(KC))
            po = [
                o_ps.tile([P, E + 1], f32, tag="ops", name=f"po{i}")
                for i in range(QS)
            ]
            resid = [f() for f in resid_tiles]
            mid_exp = None
            for i, kc in enumerate(kc_order):
                if mid_emit and i in mid_emit:
                    mid_emit[i](last_ex)
                ps = mm_ps.tile([P, QB], f32, tag="mm")
                for dc in range(DC):
                    nc.tensor.matmul(
                        ps[:],
                        lhsT=kT_sb[:, dc, kc * P:(kc + 1) * P],
                        rhs=qT_sb[:, dc, qb * QB:(qb + 1) * QB],
                        start=(dc == 0),
                        stop=(dc == DC - 1),
                    )
                ex = expp.tile([P, QB], bf16, tag="exp")
                last_ex = nc.scalar.activation(
                    out=ex[:], in_=ps[:],
                    func=mybir.ActivationFunctionType.Exp,
                    scale=float(SCALE),
                )

                for qs in range(QS):
                    nc.tensor.matmul(
                        po[qs][:],
                        lhsT=ex[:, qs * P:(qs + 1) * P],
                        rhs=v_sb[:, kc, :],
                        start=(i == 0),
                        stop=(i == KC - 1),
                    )
            ats, mvs = [], []
            var4 = stats.tile([P, QS], f32, tag="var4")
            for qs in range(QS):
                den = stats.tile([P, 1], f32, tag="den")
                nc.vector.reciprocal(out=den[:], in_=po[qs][:, E:E + 1])
                at = work.tile([P, E], f32, tag="attn", name=f"at{qs}")
                nc.vector.tensor_scalar_mul(
                    out=at[:], in0=po[qs][:, :E], scalar1=den[:]
                )
                nc.vector.tensor_add(out=at[:], in0=at[:], in1=resid[qs])
                st = stats.tile([P, nc.vector.BN_STATS_DIM], f32, tag="bst")
                nc.vector.bn_stats(out=st[:], in_=at[:])
                mv = stats.tile([P, nc.vector.BN_AGGR_DIM], f32, tag="bag",
                                name=f"mv{qs}")
                nc.vector.bn_aggr(out=mv[:], in_=st[:])
                nc.vector.tensor_scalar_add(
                    out=var4[:, qs:qs + 1], in0=mv[:, 1:2], scalar1=EPS
                )
                ats.append(at)
                mvs.append(mv)
            # rstd = rsqrt(var4); Newton on DVE keeps ACT on the Exp
            # table; the final block uses ACT Sqrt (no Exp follows it).
            rstd = stats.tile([P, QS], f32, tag="rstd")
            if act_rsqrt:
                nc.scalar.activation(
                    out=rstd[:], in_=var4[:],
                    func=mybir.ActivationFunctionType.Sqrt,
                )
                nc.vector.reciprocal(out=rstd[:], in_=rstd[:])
            else:
                tmp = stats.tile([P, QS], f32, tag="nwt")
                nc.vector.reciprocal(out=rstd[:], in_=var4[:])
                for _ in range(5):
                    nc.vector.tensor_mul(out=tmp[:], in0=rstd[:], in1=rstd[:])
                    nc.vector.tensor_mul(out=tmp[:], in0=tmp[:], in1=var4[:])
                    nc.vector.tensor_scalar(
                        out=tmp[:], in0=tmp[:], scalar1=-0.5, scalar2=1.5,
                        op0=mybir.AluOpType.mult, op1=mybir.AluOpType.add,
                    )
                    nc.vector.tensor_mul(out=rstd[:], in0=rstd[:], in1=tmp[:])
            for qs in range(QS):
                at = ats[qs]
                nc.vector.tensor_scalar(
                    out=at[:], in0=at[:],
                    scalar1=mvs[qs][:, 0:1], scalar2=rstd[:, qs:qs + 1],
                    op0=mybir.AluOpType.subtract, op1=mybir.AluOpType.mult,
                )
                nc.vector.tensor_mul(out=at[:], in0=at[:], in1=g_bc[:])
                dst = out_dst(qs) if out_dst else at
                ins = nc.vector.tensor_add(out=dst[:], in0=at[:], in1=be_bc[:])
                if qs == 1:
                    mid_exp = ins
                out_cb(qs, dst)
            return mid_exp

        # ---------------- layer 1 ----------------
        xT_sb = srcT_pool.tile([P, DC, S], bf16, tag="srcT")
        kT1 = kt_pool.tile([P, DC, S], bf16, tag="kt")
        v1 = v_pool.tile([P, KC, E + 1], bf16, tag="v")
        nc.vector.memset(v1[:, :, E:E + 1], 1.0)
        qT1 = qt_pool.tile([P, DC, SQ], bf16, tag="qt")
        xT_r = xT.rearrange("p (dc s) -> p dc s", dc=DC)
        G = 1024
        for c in range(S // QB):
            if c % 2 == 0:
                g = c // 2
                nc.sync.dma_start(
                    out=xT_sb[:, :, g * G:(g + 1) * G],
                    in_=xT_r[:, :, g * G:(g + 1) * G],
                )
            if c == 2:
                _load_late_consts()
            proj_k_chunk(kT1, xT_sb, w_sb["wkt1"], bias_sb["bk1"], c, c)
            proj_v_chunk(v1, xT_sb, w_sb["wvt1"], bcast_sb["bv1"], c, c)
            if c < NQB:
                proj_q_chunk(qT1, xT_sb, w_sb["wqt1"], bias_sb["bq1"], c)

        o_tiles = []
        anchors = {}

        def emit_l2_chunk(qb, after=None):
            def _anchored(inst):
                if after is not None:
                    tile.add_dep_helper(
                        inst.ins, after.ins, sync=False,
                        reason="chunk work after previous block epilogue",
                    )
            for dc in range(DC):
                _anchored(nc.sync.dma_start_transpose(
                    out=oqT_sb[:, dc, qb * QB:(qb + 1) * QB],
                    in_=o_bounce[qb * QB:(qb + 1) * QB, dc * P:(dc + 1) * P],
                ))
            proj_q_chunk(qT2, oqT_sb, w_sb["wqt2"], bias_sb["bq2"], qb,
                         pool=p2_ps)
            for half in range(2):
                r0 = half * SQ + qb * QB
                c = half * NQB + qb
                for dc in range(DC):
                    _anchored(nc.sync.dma_start_transpose(
                        out=oT_sb[:, dc, r0:r0 + QB],
                        in_=o_chunks[qb][half * QB:(half + 1) * QB,
                                         dc * P:(dc + 1) * P],
                    ))
                proj_k_chunk(kT2, oT_sb, w_sb["wkt2"], bias_sb["bk2"], c, c,
                             pool=p2_ps)
                proj_v_chunk(v2, oT_sb, w_sb["wvt2"], bcast_sb["bv2"], c, c,
                             pool=p2_ps)

        def make_resid1(qb, qs):
            def f():
                t = work.tile([P, E], f32, tag="xq")
                nc.sync.dma_start(
                    out=t[:], in_=xq[(qb * QS + qs) * P:(qb * QS + qs + 1) * P, :]
                )
                return t
            return f

        # layer-2 destination tiles (written chunk-by-chunk as AllGather
        # results land, interleaved with the remaining layer-1 attention)
        oT_sb = srcT_pool.tile([P, DC, S], bf16, tag="oT")
        oqT_sb = qt_pool.tile([P, DC, SQ], bf16, tag="oqT")
        kT2 = kt_pool.tile([P, DC, S], bf16, tag="kt")
        v2 = v_pool.tile([P, KC, E + 1], bf16, tag="v")
        nc.vector.memset(v2[:, :, E:E + 1], 1.0)
        qT2 = qt_pool.tile([P, DC, SQ], bf16, tag="qt")

        for qb in range(NQB):
            def dst1(qs):
                ot = o_pool.tile([P, E], f32, tag="okeep")
                o_tiles.append(ot)
                return ot

            def out1(qs, ot, qb=qb):
                ob = work.tile([P, E], bf16, tag="obf")
                nc.vector.tensor_copy(out=ob[:], in_=ot[:])
                r = (qb * QS + qs) * P
                nc.sync.dma_start(out=o_bounce[r:r + P, :], in_=ob[:])

            anchors[qb] = attention_block(
                kT1, v1, qT1, qb,
                [make_resid1(qb, qs) for qs in range(QS)],
                bcast_sb["g1"], bcast_sb["be1"], out1, out_dst=dst1,
            )

            # exchange this block's LN1 rows within the batch pair.
            nc.gpsimd.collective_compute(
                "AllGather",
                mybir.AluOpType.bypass,
                ins=[o_bounce[qb * QB:(qb + 1) * QB, :].opt()],
                outs=[o_chunks[qb].opt()],
                replica_groups=[[0, 1], [2, 3], [4, 5], [6, 7]],
            )

            # layer-2 chunk work for the PREVIOUS block: its AllGather had a
            # full attention block of time to land, so these never stall the
            # static per-engine instruction streams.
            if qb > 0:
                emit_l2_chunk(qb - 1, after=anchors[qb])

        # key chunks ordered by AllGather-chunk arrival: {0,4},{1,5},{2,6},{3,7}
        kc_order2 = [
            kc
            for cc in [0, 4, 1, 5, 2, 6, 3, 7]
            for kc in range(cc * (QB // P), (cc + 1) * (QB // P))
        ]
        for qb in range(NQB):
            def out2(qs, at, qb=qb):
                r = (qb * QS + qs) * P
                nc.sync.dma_start(out=y[r:r + P, :], in_=at[:])

            attention_block(
                kT2, v2, qT2, qb,
                [
                    (lambda qs=qs, qb=qb: o_tiles[qb * QS + qs])
                    for qs in range(QS)
                ],
                bcast_sb["g2"], bcast_sb["be2"], out2,
                kc_order=kc_order2,
                act_rsqrt=(qb == NQB - 1),
                mid_emit=(
                    {12: (lambda a: emit_l2_chunk(NQB - 1, after=a))}
                    if qb == 0 else None
                ),
            )


def _prep_inputs(x, Wq1, bq1, Wk1, bk1, Wv1, bv1, Wq2, bq2, Wk2, bk2, Wv2,
                 bv2, g1, beta1, g2, beta2):
    bfl = ml_dtypes.bfloat16
    shared = {}
    def _stripe(a2d):
        e_in, n = a2d.shape
        return np.ascontiguousarray(
            a2d.reshape(DC, P, n).transpose(1, 0, 2).reshape(P, DC * n)
        )

    for n, w in [("wqt1", Wq1), ("wkt1", Wk1), ("wvt1", Wv1),
                 ("wqt2", Wq2), ("wkt2", Wk2), ("wvt2", Wv2)]:
        shared[n] = _stripe(np.asarray(w, np.float32).T.astype(bfl))
    for n, v in [("bq1", bq1), ("bk1", bk1), ("bv1", bv1),
                 ("bq2", bq2), ("bk2", bk2), ("bv2", bv2),
                 ("g1", g1), ("be1", beta1), ("g2", g2), ("be2", beta2)]:
        shared[n] = np.ascontiguousarray(np.asarray(v, np.float32))

    x = np.asarray(x, np.float32)
    in_maps = []
    for c in range(N_CORES):
        b, h = c // 2, c % 2
        xb = x[b]
        if h:
            xb = np.concatenate([xb[SQ:], xb[:SQ]], axis=0)
        m = dict(shared)
        m["xT"] = _stripe(np.ascontiguousarray(xb.T).astype(bfl))
        m["xq"] = np.ascontiguousarray(xb[:SQ])
        in_maps.append(m)
    return in_maps


def _get_compiled():
    global _COMPILED
    if _COMPILED is None:
        _COMPILED = _build()
    return _COMPILED


def run(trace=False, **inputs):
    nc = _get_compiled()
    in_maps = _prep_inputs(**inputs)
    last_err = None
    for _ in range(3):
        try:
            res = bass_utils.run_bass_kernel_spmd(
                nc, in_maps, core_ids=list(range(N_CORES)), trace=trace
            )
            break
        except Exception as e:  # transient NRT device errors; retry
            last_err = e
    else:
        raise last_err
    out = np.empty((B, S, E), np.float32)
    for c in range(N_CORES):
        b, h = c // 2, c % 2
        out[b, h * SQ:(h + 1) * SQ] = res.results[c]["y"]
    return out, res


def kernel(**inputs):
    out, _ = run(trace=False, **inputs)
    return out
